# revision 35
# baseline (speedup 1.0000x reference)
"""Channel-attention module (CAM) forward for Trainium2.

Computes, per batch b:
    f1 = x[b].reshape(C, H*W)                      # [512, 4096]
    S  = f1 @ f1.T                                 # [512, 512] (symmetric)
    G  = softmax(S_max - S, axis=-1)
    fc = G @ f1
    y[b] = beta * fc + x[b]

Sharding: data-parallel over batch B=16 across 8 NeuronCores (2/core).

Structure (v7 — asymmetric transpose provisioning, per-row softmax shift):
  - The S matmuls need f1^T (contraction over n on partitions). Batch 0's
    x16 arrives first and the vector engines are idle early, so b0 builds
    f1^T on-device (PE transpose-mode matmuls into PSUM at fp8 step-2 +
    one bitcast-f16 drain per 2-k-tile group). Batch 1's x16 arrives last
    (nothing can hide its prep), so the host uploads its f1^T pre-cast to
    fp8 packed ("xt", 2MB): xt[h,p,k,c] = fp8(x16[1,c,h*2048+k*128+p]).
    Total HBM traffic: 18MB/core — the DMA stream is the bottleneck and
    runs gap-free; every compute phase hides behind it.
  - Per-row softmax shift: E[d,:] = exp(s_d - S[d,:]) with s_d = row-min,
    so each m-tile's exp fires right after its own row-min (no global-min
    chain). fc via S's symmetry: out[c,n] = sum_d E[d,c] f1[d,n] =
    sum_d e^{s_d} e^{-S[c,d]} f1[d,n]; the row weights e^{s_d} cancel
    against the column-sum normalizer Zcol[c] = sum_d E[d,c] (16 tiny
    ones-vector matmuls on the PE), so y = (beta/Zcol[c])*fc_raw[c,:] + x
    is the exact reference softmax.
  - fp8e4 matmuls in DoubleRow perf mode for S and fc; fp32 PSUM.
  - f8q (fc rhs + b0's transpose source) is cast on-device from x16 in
    [128,1024] pieces scheduled to each engine's idle windows.
  - beta-robust: Zcol is clamped before the reciprocal and br4 is
    written through a beta!=0-predicated copy, so beta=0 yields exactly
    y = x even if a degenerate row overflowed the softmax normalizer.
  - All HBM DMA issues from the SP sequencer via HWDGE. Queue order:
    [x16 b0 (ct,half)][beta][xt b1][x16 b1][stores b0][stores b1].
    PE order: warmup, tg0/S0 waves, fc0 (fc-pool only, qe-inner),
    S1 (packed, slotted before fc0's last quarter), fc1 (qe-split over
    all 8 banks).
"""

import numpy as np

B, C, HW = 16, 512, 4096
NCORES = 8
BL = B // NCORES  # batches per core
P = 128
CT = C // P       # 4 c-tiles of 128 channels
F = 512           # psum free dim / fc n-chunk
NQ = HW // 4      # 1024: store/cast granularity
HALF = HW // 2    # 2048
KTH = 16          # k-tiles per half

_CACHE = {}
_PHASES = []  # (label, next-instruction marker) for offline timeline analysis


def _build():
    import concourse.bass as bass  # noqa: F401
    import concourse.mybir as mybir
    import concourse.tile as tile
    from concourse import bacc
    from concourse.masks import make_identity

    f32 = mybir.dt.float32
    f16 = mybir.dt.float16
    f8 = mybir.dt.float8e4
    AF = mybir.ActivationFunctionType
    OP = mybir.AluOpType
    AX = mybir.AxisListType
    DR = mybir.MatmulPerfMode.DoubleRow

    nc = bacc.Bacc("TRN2", target_bir_lowering=False, debug=False)
    x_d = nc.dram_tensor("x", [BL, C, HW], f16, kind="ExternalInput")
    xt_d = nc.dram_tensor("xt", [2, P, KTH, F], f8, kind="ExternalInput")
    beta_d = nc.dram_tensor("beta", [1], f32, kind="ExternalInput")
    y_d = nc.dram_tensor("y", [BL, C, HW], f16, kind="ExternalOutput")

    def _mark(label):
        _PHASES.append((label, nc.get_next_instruction_name()))

    with tile.TileContext(nc) as tc:
        with (
            tc.tile_pool(name="singles", bufs=1) as singles,
            tc.tile_pool(name="x16", bufs=8) as x16_p,      # [128,4096] f16
            tc.tile_pool(name="f1t", bufs=3) as f1t_p,      # <=16KB/part
            tc.tile_pool(name="f8", bufs=4) as f8_p,        # [128,2,4096] f8
            tc.tile_pool(name="e2", bufs=4) as e2_p,        # [128,2,512] f8
            tc.tile_pool(name="soft", bufs=28) as soft_p,   # [128,<=4] f32
            tc.tile_pool(name="outs", bufs=12) as out_p,    # [128,2048] f16
            tc.tile_pool(name="tmps", bufs=8) as tmp_p,     # [128,512] f16
            tc.tile_pool(name="ps_s", bufs=4, space="PSUM") as ps_s,
            tc.tile_pool(name="ps_fc", bufs=4, space="PSUM") as ps_fc,
        ):
            beta_sb = singles.tile([P, 1], f32)
            beta_nz = singles.tile([P, 1], mybir.dt.uint8)
            ones8 = singles.tile([P, 1], f8)
            nc.gpsimd.memset(ones8[:], 1.0)
            ident = singles.tile([P, P], f8)
            make_identity(nc, ident[:])

            state = {}

            def new_state(b):
                # b0's f1t keeps fp8 values in the low byte of 2-byte cells
                # (the PE transpose writes fp8 at element step 2); b1's is
                # packed fp8 straight from HBM
                shape = [P, KTH, F, 2] if b == 0 else [P, KTH, F]
                state[b] = {
                    "x16": [
                        x16_p.tile([P, HW], f16, tag="x16", name=f"x_{b}_{ct}")
                        for ct in range(CT)
                    ],
                    "f8q": [
                        f8_p.tile([P, 2, HW], f8, tag="f8", name=f"f8_{b}_{q}")
                        for q in range(2)
                    ],
                    "f1t": [
                        f1t_p.tile(shape, f8, tag="f1t", name=f"f1t_{b}_{h}")
                        for h in range(2)
                    ],
                    "s_ps": [
                        ps_s.tile([P, F], f32, tag="s", name=f"s_ps_{b}_{m}")
                        for m in range(CT)
                    ],
                    "zps": ps_s.tile([P, F], f32, tag="s", name=f"zps_{b}"),
                    "br4": soft_p.tile([P, CT], f32, tag="sm", name=f"br4_{b}"),
                }
                nc.gpsimd.memset(state[b]["br4"][:], 0.0)

            def s_lhsT(b, h, q, m):
                f1t = state[b]["f1t"][h]
                if b == 0:
                    return f1t[:, 2 * q : 2 * q + 2, m * P : (m + 1) * P, 0]
                return f1t[:, 2 * q : 2 * q + 2, m * P : (m + 1) * P]

            def s_rhs(b, h, q):
                f1t = state[b]["f1t"][h]
                if b == 0:
                    return f1t[:, 2 * q : 2 * q + 2, :, 0]
                return f1t[:, 2 * q : 2 * q + 2, :]

            def emit_loads_b0():
                _mark('loads0')
                # (ct, half) granularity so cast/transpose waves start as
                # soon as each half's c-tiles land
                x16 = state[0]["x16"]
                for h in range(2):
                    for ct in range(CT):
                        nc.sync.dma_start(
                            out=x16[ct][:, h * HALF : (h + 1) * HALF],
                            in_=x_d[0, ct * P : (ct + 1) * P, h * HALF : (h + 1) * HALF],
                        )

            def emit_loads_b1():
                _mark('loads1')
                f1t, x16 = state[1]["f1t"], state[1]["x16"]
                for h in range(2):
                    for ks in range(2):
                        nc.sync.dma_start(
                            out=f1t[h][:, ks * 8 : (ks + 1) * 8, :],
                            in_=xt_d[h, :, ks * 8 : (ks + 1) * 8, :],
                        )
                for ct in range(CT):
                    nc.sync.dma_start(
                        out=x16[ct][:], in_=x_d[1, ct * P : (ct + 1) * P, :]
                    )

            def emit_casts(b, cts, engs, qs=(0, 1, 2, 3)):
                _mark(f'cast{b}_{cts[0]}_{qs[0]}')
                # fp16 -> fp8 casts in [128,1024] pieces on the given engines
                x16, f8q = state[b]["x16"], state[b]["f8q"]
                i = 0
                for ct in cts:
                    for q in qs:
                        dst = f8q[ct // 2][:, ct % 2, q * NQ : (q + 1) * NQ]
                        src = x16[ct][:, q * NQ : (q + 1) * NQ]
                        engs[i % len(engs)](out=dst, in_=src)
                        i += 1

            def emit_casts_q(b, q, engs):
                _mark(f'castq{b}_{q}')
                # quarter-major: the 4 c-tile pieces of quarter q (unblocks
                # transpose groups 4q..4q+3)
                x16, f8q = state[b]["x16"], state[b]["f8q"]
                for i, ct in enumerate(range(CT)):
                    dst = f8q[ct // 2][:, ct % 2, q * NQ : (q + 1) * NQ]
                    src = x16[ct][:, q * NQ : (q + 1) * NQ]
                    engs[i % len(engs)](out=dst, in_=src)

            def emit_tgroup(b, g, copy_eng):
                _mark(f'tg{b}_{g}')
                # transpose group g: 2 k-tiles x 4 c-tiles = 8 fp8 PE
                # transposes into one PSUM bank. fp8 transpose outputs land
                # at element step 2; the drain is one dense bitcast-f16
                # copy (2x DVE mode) and S reads the even bytes via step-2
                # APs.
                f8q, f1t = state[b]["f8q"], state[b]["f1t"]
                h, gl = g // 8, g % 8
                xp = ps_fc.tile([P, 8, P, 2], f8, tag="fc", name=f"xp_{b}_{g}")
                for i in range(8):
                    kl, ct = gl * 2 + i // 4, i % 4
                    nc.tensor.transpose(
                        xp[:, i, :, 0],
                        f8q[ct // 2][
                            :, ct % 2, h * HALF + kl * P : h * HALF + (kl + 1) * P
                        ],
                        ident[:],
                    )
                dst = f1t[h][:, gl * 2 : gl * 2 + 2, :, :].bitcast(f16)
                copy_eng(out=dst, in_=xp[:].bitcast(f16))

            def emit_S_h0(b):
                _mark(f'Sh0_{b}')
                s_ps = state[b]["s_ps"]
                for q in range(8):
                    for m in range(CT):
                        nc.tensor.matmul(
                            s_ps[m][:],
                            lhsT=s_lhsT(b, 0, q, m),
                            rhs=s_rhs(b, 0, q),
                            start=(q == 0),
                            stop=False,
                            perf_mode=DR,
                        )

            def emit_S_h1_stats(b):
                _mark(f'Sh1_{b}')
                # m-outer; each row-min fires as its m-tile stops and feeds
                # that m-tile's exp directly (per-row shift)
                s_ps = state[b]["s_ps"]
                nm4 = soft_p.tile([P, CT], f32, tag="sm", name=f"nm4_{b}")
                for m in range(CT):
                    for q in range(8):
                        nc.tensor.matmul(
                            s_ps[m][:],
                            lhsT=s_lhsT(b, 1, q, m),
                            rhs=s_rhs(b, 1, q),
                            start=False,
                            stop=(q == 7),
                            perf_mode=DR,
                        )
                    nc.vector.tensor_reduce(
                        out=nm4[:, m : m + 1],
                        in_=s_ps[m][:],
                        axis=AX.X,
                        op=OP.min,
                    )
                state[b]["nm4"] = nm4

            def emit_exps(b):
                _mark(f'exp{b}')
                # E[d,:] = exp(s_d - S[d,:]) in fp8; each exp needs only its
                # own m-tile's row-min, so the chain pipelines with S h1
                s_ps, nm4 = state[b]["s_ps"], state[b]["nm4"]
                e2 = [
                    e2_p.tile([P, 2, F], f8, tag="e2", name=f"e2_{b}_{g}")
                    for g in range(2)
                ]
                for m in range(CT):
                    nc.scalar.activation(
                        out=e2[m // 2][:, m % 2, :],
                        in_=s_ps[m][:],
                        func=AF.Exp,
                        bias=nm4[:, m : m + 1],
                        scale=-1.0,
                    )
                state[b]["e2"] = e2

            def emit_zcol_br(b):
                _mark(f'br{b}')
                # Zcol[c] = sum_d E[d,c] via 16 tiny ones-vector matmuls
                # (free size 1), accumulated per c-block in PSUM; then
                # beta/Zcol, finite for any beta: clamp before the
                # reciprocal and write br4 beta-predicated so beta=0 gives
                # exactly 0.0
                e2, zps = state[b]["e2"], state[b]["zps"]
                for m in range(CT):
                    for db in range(CT):
                        nc.tensor.matmul(
                            zps[:, m : m + 1],
                            lhsT=e2[db // 2][:, db % 2, m * P : (m + 1) * P],
                            rhs=ones8[:],
                            start=(db == 0),
                            stop=(db == 3),
                        )
                zs = soft_p.tile([P, CT], f32, tag="sm", name=f"zs_{b}")
                nc.vector.tensor_scalar(
                    out=zs[:], in0=zps[:, 0:CT], scalar1=1e-35, scalar2=None,
                    op0=OP.max,
                )
                rz = soft_p.tile([P, CT], f32, tag="sm", name=f"rz_{b}")
                nc.vector.reciprocal(out=rz[:], in_=zs[:])
                rzb = soft_p.tile([P, CT], f32, tag="sm", name=f"rzb_{b}")
                nc.gpsimd.tensor_scalar_mul(out=rzb[:], in0=rz[:], scalar1=beta_sb[:])
                br4 = state[b]["br4"]
                nc.vector.copy_predicated(
                    out=br4[:],
                    mask=beta_nz[:].broadcast_to([P, CT]),
                    data=rzb[:],
                )

            def emit_fc_quarter(b, nq, cnt, ot):
                _mark(f'fc{b}_{nq}')
                # fc_raw = E @ f1 for quarter nq; qe=0 sweep first (needs
                # only exps 0-1 / f8q c-tiles 0-1) when 8 banks are free
                # (b1), else chunk-complete qe-inner on the 4-bank fc pool
                # (b0 — the s-pool slots carry S/Z state for both batches).
                # Fused epilogue y = (beta/Zcol)[c]*fc_raw + x, then store.
                x16, f8q = state[b]["x16"], state[b]["f8q"]
                e2, br4 = state[b]["e2"], state[b]["br4"]
                h, qo = nq // 2, (nq % 2) * NQ
                chunks = []
                for m in range(CT):
                    if nq % 2 == 0:
                        ot[(m, h)] = out_p.tile(
                            [P, HALF], f16, tag="out", name=f"ot_{b}_{h}_{m}"
                        )
                    for jj in range(2):
                        j = nq * 2 + jj
                        c = cnt[0]
                        use_s = (b == 1) and (c % 2 == 1)
                        pool = ps_s if use_s else ps_fc
                        tag = "s" if use_s else "fc"
                        f_ps = pool.tile([P, F], f32, tag=tag, name=f"f_{b}_{j}_{m}")
                        chunks.append((m, jj, j, c, f_ps))
                        cnt[0] += 1
                if b == 1:
                    for qe in range(2):
                        for m, jj, j, c, f_ps in chunks:
                            nc.tensor.matmul(
                                f_ps[:],
                                lhsT=e2[qe][:, :, m * P : (m + 1) * P],
                                rhs=f8q[qe][:, :, j * F : (j + 1) * F],
                                start=(qe == 0),
                                stop=(qe == 1),
                                perf_mode=DR,
                            )
                else:
                    for m, jj, j, c, f_ps in chunks:
                        for qe in range(2):
                            nc.tensor.matmul(
                                f_ps[:],
                                lhsT=e2[qe][:, :, m * P : (m + 1) * P],
                                rhs=f8q[qe][:, :, j * F : (j + 1) * F],
                                start=(qe == 0),
                                stop=(qe == 1),
                                perf_mode=DR,
                            )
                for m, jj, j, c, f_ps in chunks:
                    o = ot[(m, h)]
                    oslice = o[:, qo + jj * F : qo + (jj + 1) * F]
                    xslice = x16[m][:, nq * NQ + jj * F : nq * NQ + (jj + 1) * F]
                    kind = (0, 2, 0, 1)[c % 4]
                    if kind:
                        # PSUM is only readable by ACT/DVE: ACT scales fc
                        # out of PSUM, the fp16 residual add runs on DVE
                        # (2x) or Pool (all-SBUF)
                        tmp = tmp_p.tile([P, F], f16, tag="tmp", name=f"tp_{b}_{j}_{m}")
                        nc.scalar.mul(out=tmp[:], in_=f_ps[:], mul=br4[:, m : m + 1])
                        aeng = nc.vector if kind == 1 else nc.gpsimd
                        aeng.tensor_tensor(
                            out=oslice, in0=tmp[:], in1=xslice, op=OP.add
                        )
                    else:
                        nc.vector.scalar_tensor_tensor(
                            out=oslice,
                            in0=f_ps[:],
                            scalar=br4[:, m : m + 1],
                            in1=xslice,
                            op0=OP.mult,
                            op1=OP.add,
                        )
                    if jj == 1:
                        nc.sync.dma_start(
                            out=y_d[b, m * P : (m + 1) * P, nq * NQ : (nq + 1) * NQ],
                            in_=o[:, qo : qo + NQ],
                        )

            # ---- program order / DMA queue: [x16 b0][beta][xt b1][x16 b1]
            # ---- [stores b0][stores b1]
            new_state(0)
            emit_loads_b0()
            nc.sync.dma_start(out=beta_sb[:], in_=beta_d[:].to_broadcast([P, 1]))
            nc.vector.tensor_scalar(
                out=beta_nz[:], in0=beta_sb[:], scalar1=0.0, scalar2=None,
                op0=OP.not_equal,
            )
            new_state(1)
            emit_loads_b1()
            # PE p-state pre-warm: ungated dummy matmuls run back-to-back
            # from t~0.4us so the first transposes hit full clock
            _mark('warm')
            wsrc = singles.tile([P, 2, F], f8)
            nc.gpsimd.memset(wsrc[:], 0.0)
            wps = ps_fc.tile([P, F], f32, tag="fc", name="warm_ps")
            for i in range(16):
                nc.tensor.matmul(
                    wps[:],
                    lhsT=wsrc[:, :, 0:P],
                    rhs=wsrc[:],
                    start=(i == 0),
                    stop=(i == 15),
                    perf_mode=DR,
                )
            A, V, G = nc.scalar.copy, nc.vector.tensor_copy, nc.gpsimd.tensor_copy
            # b0 h0: cast waves (quarter-major) -> transpose groups -> S h0
            emit_casts_q(0, 0, [A, V, A, V])
            emit_casts_q(0, 1, [V, A, V, A])
            for g in range(8):
                emit_tgroup(0, g, V if g % 2 == 0 else A)
            emit_S_h0(0)
            # b0 h1: Pool takes half the cast pieces (ACT/DVE carry the
            # drains); drains lean DVE
            emit_casts_q(0, 2, [G, V, G, V])
            emit_casts_q(0, 3, [V, G, V, G])
            for g in range(8, 16):
                emit_tgroup(0, g, A if g in (10, 14) else V)
            emit_S_h1_stats(0)
            emit_exps(0)
            emit_zcol_br(0)
            cnt0, ot0 = [0], {}
            emit_fc_quarter(0, 0, cnt0, ot0)
            emit_fc_quarter(0, 1, cnt0, ot0)
            # S(1) h0 early: its matmuls fill the PE stalls of fc0's
            # epilogue-paced bank recycling
            emit_S_h0(1)
            emit_fc_quarter(0, 2, cnt0, ot0)
            emit_fc_quarter(0, 3, cnt0, ot0)
            # b1 casts follow the x16-b1 c-tile arrivals: Pool/ACT take
            # ct0/1 (ACT pieces finish before exps(1) needs it), DVE picks
            # up ct2/3 after the row-min rail
            emit_casts(1, (0,), [G, A, G, A])
            emit_casts(1, (1,), [A, G, G, G])
            emit_S_h1_stats(1)
            emit_casts(1, (2,), [V, V, G, V])
            emit_casts(1, (3,), [V, V, G], qs=(0, 1, 2))
            emit_exps(1)
            emit_zcol_br(1)
            emit_casts(1, (3,), [A], qs=(3,))
            cnt1, ot1 = [0], {}
            for nq in range(4):
                emit_fc_quarter(1, nq, cnt1, ot1)
    nc.finalize()
    return nc


def _get_nc():
    if "nc" not in _CACHE:
        _CACHE["nc"] = _build()
    return _CACHE["nc"]


def kernel(x: np.ndarray, beta: np.ndarray, **kw) -> np.ndarray:
    import concourse.mybir as mybir
    from concourse.bass_utils import run_bass_kernel_spmd

    x = np.asarray(x)
    beta = np.ascontiguousarray(np.asarray(beta, dtype=np.float32))
    assert x.shape == (B, C, 64, 64), x.shape

    f8np = mybir.dt.np(mybir.dt.float8e4)
    x16 = np.ascontiguousarray(x.reshape(B, C, HW).astype(np.float16))
    # xt[b, h, p, k, c] = fp8(x16[b, c, h*2048 + k*128 + p]); only the
    # second batch of each core's pair is uploaded pre-transposed
    xt = np.ascontiguousarray(
        x16.reshape(B, C, 2, KTH, P).transpose(0, 2, 4, 3, 1).astype(f8np)
    )
    in_maps = [
        {
            "x": np.ascontiguousarray(x16[i * BL : (i + 1) * BL]),
            "xt": np.ascontiguousarray(xt[i * BL + 1]),
            "beta": beta,
        }
        for i in range(NCORES)
    ]
    nc = _get_nc()
    res = run_bass_kernel_spmd(nc, in_maps, core_ids=list(range(NCORES)))
    out = np.concatenate([r["y"] for r in res.results], axis=0)
    return out.reshape(B, C, 64, 64).astype(np.float32)


# revision 37
# speedup vs baseline: 1.1653x; 1.1653x over previous
"""Channel-attention module (CAM) forward for Trainium2.

Computes, per batch b:
    f1 = x[b].reshape(C, H*W)                      # [512, 4096]
    S  = f1 @ f1.T                                 # [512, 512] (symmetric)
    G  = softmax(S_max - S, axis=-1) == exp(S_min_row - S) / rowsum
    fc = G @ f1
    y[b] = beta * fc + x[b]

Sharding: data-parallel over batch B=16 across 8 NeuronCores (2/core).

Structure (v2 — host-transposed fp8 upload):
  - The S matmuls need f1^T (contraction over n must sit on partitions).
    Instead of PE transpose-mode matmuls + PSUM bitcast drains (which made
    PE/ACT/DVE the bottleneck), the host uploads f1^T pre-cast to fp8
    ("xt", 2MB/batch): xt[b, h, p, k, c] = fp8(x[b, c, h*2048 + k*128 + p]).
    This removes ~14us of PE transpose work and ~27us of ACT/DVE drain
    work per core for +11.7us of DMA; the kernel becomes DMA-bound at
    ~58us of traffic (4MB x16 + 2MB xt in, 4MB y out, per batch).
  - Global-shift symmetric exp: E = exp(s0 - S) with one scalar s0
    (global min of S) is symmetric, so fc_raw = E @ f1 takes its matmul
    lhsT directly from E's stored row-tiles. The per-row softmax scale
    folds into the epilogue: y = (beta / Zraw[c]) * fc_raw + x with
    Zraw = rowsum(E); the s0 and row-min shifts cancel exactly.
  - fp8e4 matmuls in DoubleRow perf mode for both S and fc; fp32 PSUM.
  - x is staged host-side to fp16; f8q (fc rhs) is cast on-device from
    x16 in [128,1024] pieces split across ACT/DVE/Pool.
  - beta-robust: Zraw is clamped before the reciprocal and br4 is
    written through a beta!=0-predicated copy, so beta=0 yields exactly
    y = x even if a degenerate row overflowed the softmax normalizer.
  - All HBM DMA issues from the SP sequencer via HWDGE. Queue order:
    [x16 b0][xt b0][x16 b1][xt b1][stores b0][stores b1] — the DMA
    stream is the bottleneck and never idles; compute hides behind it.
    PE order: S(0), fc(0), S(1), fc(1); fc alternates the S-pool and
    fc-pool PSUM banks (8 total) so bank recycling never stalls the PE.
"""

import numpy as np

B, C, HW = 16, 512, 4096
NCORES = 8
BL = B // NCORES  # batches per core
P = 128
CT = C // P       # 4 c-tiles of 128 channels
F = 512           # psum free dim / fc n-chunk
NQ = HW // 4      # 1024: store/cast granularity
HALF = HW // 2    # 2048
KTH = 16          # k-tiles per half

_CACHE = {}
_PHASES = []  # (label, next-instruction marker) for offline timeline analysis


def _build():
    import concourse.bass as bass  # noqa: F401
    import concourse.mybir as mybir
    import concourse.tile as tile
    from concourse import bacc, bass_isa

    f32 = mybir.dt.float32
    f16 = mybir.dt.float16
    f8 = mybir.dt.float8e4
    AF = mybir.ActivationFunctionType
    OP = mybir.AluOpType
    AX = mybir.AxisListType
    DR = mybir.MatmulPerfMode.DoubleRow

    nc = bacc.Bacc("TRN2", target_bir_lowering=False, debug=False)
    x_d = nc.dram_tensor("x", [BL, C, HW], f16, kind="ExternalInput")
    xt_d = nc.dram_tensor("xt", [BL, 2, P, KTH, F], f8, kind="ExternalInput")
    beta_d = nc.dram_tensor("beta", [1], f32, kind="ExternalInput")
    y_d = nc.dram_tensor("y", [BL, C, HW], f16, kind="ExternalOutput")

    def _mark(label):
        _PHASES.append((label, nc.get_next_instruction_name()))

    with tile.TileContext(nc) as tc:
        with (
            tc.tile_pool(name="singles", bufs=1) as singles,
            tc.tile_pool(name="x16", bufs=8) as x16_p,      # [128,4096] f16
            tc.tile_pool(name="f1t", bufs=4) as f1t_p,      # [128,16,512] f8
            tc.tile_pool(name="f8", bufs=4) as f8_p,        # [128,2,4096] f8
            tc.tile_pool(name="e2", bufs=4) as e2_p,        # [128,2,512] f8
            tc.tile_pool(name="soft", bufs=28) as soft_p,   # [128,<=4] f32
            tc.tile_pool(name="outs", bufs=16) as out_p,    # [128,2048] f16
            tc.tile_pool(name="tmps", bufs=8) as tmp_p,     # [128,512] f16
            tc.tile_pool(name="ps_s", bufs=4, space="PSUM") as ps_s,
            tc.tile_pool(name="ps_fc", bufs=4, space="PSUM") as ps_fc,
        ):
            beta_sb = singles.tile([P, 1], f32)
            beta_nz = singles.tile([P, 1], mybir.dt.uint8)
            ones8 = singles.tile([P, 1], f8)
            nc.gpsimd.memset(ones8[:], 1.0)

            state = {}

            def new_state(b):
                state[b] = {
                    "x16": [
                        x16_p.tile([P, HW], f16, tag="x16", name=f"x_{b}_{ct}")
                        for ct in range(CT)
                    ],
                    "f8q": [
                        f8_p.tile([P, 2, HW], f8, tag="f8", name=f"f8_{b}_{q}")
                        for q in range(2)
                    ],
                    "f1t": [
                        f1t_p.tile([P, KTH, F], f8, tag="f1t", name=f"f1t_{b}_{h}")
                        for h in range(2)
                    ],
                    "s_ps": [
                        ps_s.tile([P, F], f32, tag="s", name=f"s_ps_{b}_{m}")
                        for m in range(CT)
                    ],
                    "br4": soft_p.tile([P, CT], f32, tag="sm", name=f"br4_{b}"),
                }
                nc.gpsimd.memset(state[b]["br4"][:], 0.0)

            def emit_loads(b):
                _mark(f'loads{b}')
                # f1t k-slabs first (gate S), then x16 (casts/residual)
                x16, f1t = state[b]["x16"], state[b]["f1t"]
                for h in range(2):
                    for ks in range(2):
                        nc.sync.dma_start(
                            out=f1t[h][:, ks * 8 : (ks + 1) * 8, :],
                            in_=xt_d[b, h, :, ks * 8 : (ks + 1) * 8, :],
                        )
                for ct in range(CT):
                    nc.sync.dma_start(
                        out=x16[ct][:], in_=x_d[b, ct * P : (ct + 1) * P, :]
                    )

            def emit_casts(b, cts, engs, qs=(0, 1, 2, 3)):
                _mark(f'cast{b}_{cts[0]}')
                # fp16 -> fp8 casts in [128,1024] pieces on the given engines
                x16, f8q = state[b]["x16"], state[b]["f8q"]
                i = 0
                for ct in cts:
                    for q in qs:
                        dst = f8q[ct // 2][:, ct % 2, q * NQ : (q + 1) * NQ]
                        src = x16[ct][:, q * NQ : (q + 1) * NQ]
                        engs[i % len(engs)](out=dst, in_=src)
                        i += 1

            def emit_S_h0(b):
                _mark(f'Sh0_{b}')
                f1t, s_ps = state[b]["f1t"], state[b]["s_ps"]
                for q in range(8):
                    for m in range(CT):
                        nc.tensor.matmul(
                            s_ps[m][:],
                            lhsT=f1t[0][:, 2 * q : 2 * q + 2, m * P : (m + 1) * P],
                            rhs=f1t[0][:, 2 * q : 2 * q + 2, :],
                            start=(q == 0),
                            stop=False,
                            perf_mode=DR,
                        )

            def emit_S_h1_stats(b):
                _mark(f'Sh1_{b}')
                # m-outer; each row-min fires as its m-tile stops and feeds
                # that m-tile's exp directly (per-row shift — no global-min
                # chain; the row weights e^{s_d} cancel between fc_raw and
                # the column-sum normalizer Zcol)
                f1t, s_ps = state[b]["f1t"], state[b]["s_ps"]
                nm4 = soft_p.tile([P, CT], f32, tag="sm", name=f"nm4_{b}")
                for m in range(CT):
                    for q in range(8):
                        nc.tensor.matmul(
                            s_ps[m][:],
                            lhsT=f1t[1][:, 2 * q : 2 * q + 2, m * P : (m + 1) * P],
                            rhs=f1t[1][:, 2 * q : 2 * q + 2, :],
                            start=False,
                            stop=(q == 7),
                            perf_mode=DR,
                        )
                    nc.vector.tensor_reduce(
                        out=nm4[:, m : m + 1],
                        in_=s_ps[m][:],
                        axis=AX.X,
                        op=OP.min,
                    )
                state[b]["nm4"] = nm4

            def emit_exps(b):
                _mark(f'exp{b}')
                # E[d,:] = exp(s_d - S[d,:]) in fp8, s_d = row-min: each exp
                # needs only its own m-tile's min, so the chain pipelines
                # with the S h1 matmuls instead of waiting on a global stat
                s_ps, nm4 = state[b]["s_ps"], state[b]["nm4"]
                e2 = [
                    e2_p.tile([P, 2, F], f8, tag="e2", name=f"e2_{b}_{g}")
                    for g in range(2)
                ]
                for m in range(CT):
                    nc.scalar.activation(
                        out=e2[m // 2][:, m % 2, :],
                        in_=s_ps[m][:],
                        func=AF.Exp,
                        bias=nm4[:, m : m + 1],
                        scale=-1.0,
                    )
                state[b]["e2"] = e2

            def emit_zcol_br(b):
                _mark(f'br{b}')
                # Zcol[c] = sum_d E[d,c] via 16 tiny column-sum matmuls
                # against a ones vector (free size 1 — near-zero PE time),
                # accumulated per c-block in a [128,CT] PSUM tile. Then
                # beta / Zcol, finite for any beta: clamp before the
                # reciprocal, and write br4 through a beta-predicated copy
                # so br4 is exactly 0.0 (not 0*NaN) for beta == 0.
                e2 = state[b]["e2"]
                zps = ps_s.tile([P, F], f32, tag="s", name=f"zps_{b}")
                for m in range(CT):
                    for db in range(CT):
                        nc.tensor.matmul(
                            zps[:, m : m + 1],
                            lhsT=e2[db // 2][:, db % 2, m * P : (m + 1) * P],
                            rhs=ones8[:],
                            start=(db == 0),
                            stop=(db == 3),
                        )
                zs = soft_p.tile([P, CT], f32, tag="sm", name=f"zs_{b}")
                nc.vector.tensor_scalar(
                    out=zs[:], in0=zps[:, 0:CT], scalar1=1e-35, scalar2=None, op0=OP.max
                )
                rz = soft_p.tile([P, CT], f32, tag="sm", name=f"rz_{b}")
                nc.vector.reciprocal(out=rz[:], in_=zs[:])
                rzb = soft_p.tile([P, CT], f32, tag="sm", name=f"rzb_{b}")
                nc.gpsimd.tensor_scalar_mul(out=rzb[:], in0=rz[:], scalar1=beta_sb[:])
                br4 = state[b]["br4"]
                nc.vector.copy_predicated(
                    out=br4[:],
                    mask=beta_nz[:].broadcast_to([P, CT]),
                    data=rzb[:],
                )

            def emit_fc_quarter(b, nq, cnt, ot):
                _mark(f'fc{b}_{nq}')
                # fc_raw = E @ f1 for quarter nq; the qe=0 sweep over all 8
                # chunks runs first (needs only exps 0-1 / f8q c-tiles 0-1),
                # then the qe=1 sweep stops each chunk — pipelines fc into
                # the exp chain and the b1 cast arrivals. 8 chunks hold all
                # 8 PSUM banks (4 ps_s + 4 ps_fc); fused epilogue
                # y = (beta/Zraw)[c]*fc_raw + x follows each chunk's stop.
                x16, f8q = state[b]["x16"], state[b]["f8q"]
                e2, br4 = state[b]["e2"], state[b]["br4"]
                h, qo = nq // 2, (nq % 2) * NQ
                chunks = []
                for m in range(CT):
                    if nq % 2 == 0:
                        ot[(m, h)] = out_p.tile(
                            [P, HALF], f16, tag="out", name=f"ot_{b}_{h}_{m}"
                        )
                    for jj in range(2):
                        j = nq * 2 + jj
                        c = cnt[0]
                        # b0 front-loads its ps_s uses (and avoids them in
                        # its last two quarters) so the slots recycle to
                        # batch 1's S accumulators without stalling it
                        if b == 0 and nq >= 2:
                            use_s = False
                        else:
                            use_s = (c % 2 == 0) if b == 0 else (c % 2 == 1)
                        pool = ps_s if use_s else ps_fc
                        tag = "s" if pool is ps_s else "fc"
                        f_ps = pool.tile([P, F], f32, tag=tag, name=f"f_{b}_{j}_{m}")
                        chunks.append((m, jj, j, c, f_ps))
                        cnt[0] += 1
                # qe-split needs all 8 PSUM banks live at once; a quarter
                # restricted to one 4-bank pool must complete chunk-by-chunk
                # (qe-inner) or the in-order PE queue deadlocks on recycling
                split = not (b == 0 and nq >= 2)
                if split:
                    for qe in range(2):
                        for m, jj, j, c, f_ps in chunks:
                            nc.tensor.matmul(
                                f_ps[:],
                                lhsT=e2[qe][:, :, m * P : (m + 1) * P],
                                rhs=f8q[qe][:, :, j * F : (j + 1) * F],
                                start=(qe == 0),
                                stop=(qe == 1),
                                perf_mode=DR,
                            )
                else:
                    for m, jj, j, c, f_ps in chunks:
                        for qe in range(2):
                            nc.tensor.matmul(
                                f_ps[:],
                                lhsT=e2[qe][:, :, m * P : (m + 1) * P],
                                rhs=f8q[qe][:, :, j * F : (j + 1) * F],
                                start=(qe == 0),
                                stop=(qe == 1),
                                perf_mode=DR,
                            )
                for m, jj, j, c, f_ps in chunks:
                    o = ot[(m, h)]
                    oslice = o[:, qo + jj * F : qo + (jj + 1) * F]
                    xslice = x16[m][:, nq * NQ + jj * F : nq * NQ + (jj + 1) * F]
                    kind = (0, 2, 0, 1)[c % 4]
                    if kind:
                        # PSUM can only be read by ACT/DVE: ACT scales
                        # fc out of PSUM, the fp16 residual add runs on
                        # DVE (2x) or Pool (all-SBUF)
                        tmp = tmp_p.tile([P, F], f16, tag="tmp", name=f"tp_{b}_{j}_{m}")
                        nc.scalar.mul(out=tmp[:], in_=f_ps[:], mul=br4[:, m : m + 1])
                        aeng = nc.vector if kind == 1 else nc.gpsimd
                        aeng.tensor_tensor(
                            out=oslice, in0=tmp[:], in1=xslice, op=OP.add
                        )
                    else:
                        nc.vector.scalar_tensor_tensor(
                            out=oslice,
                            in0=f_ps[:],
                            scalar=br4[:, m : m + 1],
                            in1=xslice,
                            op0=OP.mult,
                            op1=OP.add,
                        )
                    if jj == 1:
                        if b == 1 and nq == 3 and m == 3:
                            for v2 in range(2):
                                nc.sync.dma_start(
                                    out=y_d[
                                        b, m * P : (m + 1) * P,
                                        nq * NQ + v2 * F : nq * NQ + (v2 + 1) * F,
                                    ],
                                    in_=o[:, qo + v2 * F : qo + (v2 + 1) * F],
                                )
                        else:
                            nc.sync.dma_start(
                                out=y_d[b, m * P : (m + 1) * P, nq * NQ : (nq + 1) * NQ],
                                in_=o[:, qo : qo + NQ],
                            )

            # ---- program order: DMA queue [xt b0][x16 b0][xt b1][x16 b1]
            # ---- [stores b0][stores b1]; PE queue [warmup][S0][fc0][S1][fc1]
            new_state(0)
            emit_loads(0)
            # beta rides the DMA queue behind b0's first slabs (tiny; keeps
            # the first data transfer at the queue head)
            nc.sync.dma_start(out=beta_sb[:], in_=beta_d[:].to_broadcast([P, 1]))
            nc.vector.tensor_scalar(
                out=beta_nz[:], in0=beta_sb[:], scalar1=0.0, scalar2=None,
                op0=OP.not_equal,
            )
            new_state(1)
            emit_loads(1)
            # PE p-state pre-warm: ungated dummy matmuls run back-to-back
            # from t~0.4us so S(0) starts at full clock when its first f1t
            # slab lands (~4us). The dummy reads a memset tile; its PSUM
            # slot recycles into the first fc chunks much later.
            _mark('warm')
            wsrc = singles.tile([P, 2, F], f8)
            nc.gpsimd.memset(wsrc[:], 0.0)
            wps = ps_fc.tile([P, F], f32, tag="fc", name="warm_ps")
            for i in range(16):
                nc.tensor.matmul(
                    wps[:],
                    lhsT=wsrc[:, :, 0:P],
                    rhs=wsrc[:],
                    start=(i == 0),
                    stop=(i == 15),
                    perf_mode=DR,
                )
            # b0: stats chain ahead of casts on DVE (row-mins must not sit
            # behind cast pieces gated on later x16 c-tile arrivals)
            emit_S_h0(0)
            emit_S_h1_stats(0)
            emit_casts(0, (0, 1), [nc.vector.tensor_copy, nc.gpsimd.tensor_copy])
            emit_exps(0)
            emit_zcol_br(0)
            emit_casts(
                0, (2, 3),
                [nc.scalar.copy, nc.vector.tensor_copy,
                 nc.gpsimd.tensor_copy, nc.scalar.copy],
            )
            cnt0, ot0 = [0], {}
            emit_fc_quarter(0, 0, cnt0, ot0)
            emit_fc_quarter(0, 1, cnt0, ot0)
            emit_fc_quarter(0, 2, cnt0, ot0)
            # S(1)-h0 emitted before fc0's last quarter: its matmuls fill
            # the PE stalls left by fc0's epilogue-paced bank recycling
            # (the s_ps-b1 slots are free once exps(0) has read them)
            emit_S_h0(1)
            emit_fc_quarter(0, 3, cnt0, ot0)
            # b1 casts follow the x16-b1 c-tile arrivals: ACT/Pool take
            # ct0/1 (DVE still owns fc0's epilogue, then the row-min rail),
            # DVE picks up ct2/3 after the mins; exps(1) slots before the
            # last ACT piece
            A, V, G = nc.scalar.copy, nc.vector.tensor_copy, nc.gpsimd.tensor_copy
            emit_casts(1, (0,), [A, G, A, G])
            emit_casts(1, (1,), [A, G, A, G])
            emit_S_h1_stats(1)
            emit_casts(1, (2,), [V, V, G, A])
            emit_casts(1, (3,), [V, V, G], qs=(0, 1, 2))
            emit_exps(1)
            emit_zcol_br(1)
            emit_casts(1, (3,), [A], qs=(3,))
            cnt1, ot1 = [0], {}
            for nq in range(4):
                emit_fc_quarter(1, nq, cnt1, ot1)
    nc.finalize()
    return nc


def _get_nc():
    if "nc" not in _CACHE:
        _CACHE["nc"] = _build()
    return _CACHE["nc"]


def kernel(x: np.ndarray, beta: np.ndarray, **kw) -> np.ndarray:
    import concourse.mybir as mybir
    from concourse.bass_utils import run_bass_kernel_spmd

    x = np.asarray(x)
    beta = np.ascontiguousarray(np.asarray(beta, dtype=np.float32))
    assert x.shape == (B, C, 64, 64), x.shape

    f8np = mybir.dt.np(mybir.dt.float8e4)
    x16 = np.ascontiguousarray(x.reshape(B, C, HW).astype(np.float16))
    # xt[b, h, p, k, c] = fp8(x16[b, c, h*2048 + k*128 + p])
    xt = np.ascontiguousarray(
        x16.reshape(B, C, 2, KTH, P).transpose(0, 2, 4, 3, 1).astype(f8np)
    )
    in_maps = [
        {
            "x": np.ascontiguousarray(x16[i * BL : (i + 1) * BL]),
            "xt": np.ascontiguousarray(xt[i * BL : (i + 1) * BL]),
            "beta": beta,
        }
        for i in range(NCORES)
    ]
    nc = _get_nc()
    res = run_bass_kernel_spmd(nc, in_maps, core_ids=list(range(NCORES)))
    out = np.concatenate([r["y"] for r in res.results], axis=0)
    return out.reshape(B, C, 64, 64).astype(np.float32)


# revision 38
# speedup vs baseline: 1.1662x; 1.0008x over previous
"""Channel-attention module (CAM) forward for Trainium2.

Computes, per batch b:
    f1 = x[b].reshape(C, H*W)                      # [512, 4096]
    S  = f1 @ f1.T                                 # [512, 512] (symmetric)
    G  = softmax(S_max - S, axis=-1) == exp(S_min_row - S) / rowsum
    fc = G @ f1
    y[b] = beta * fc + x[b]

Sharding: data-parallel over batch B=16 across 8 NeuronCores (2/core).

Structure (v2 — host-transposed fp8 upload):
  - The S matmuls need f1^T (contraction over n must sit on partitions).
    Instead of PE transpose-mode matmuls + PSUM bitcast drains (which made
    PE/ACT/DVE the bottleneck), the host uploads f1^T pre-cast to fp8
    ("xt", 2MB/batch): xt[b, h, p, k, c] = fp8(x[b, c, h*2048 + k*128 + p]).
    This removes ~14us of PE transpose work and ~27us of ACT/DVE drain
    work per core for +11.7us of DMA; the kernel becomes DMA-bound at
    ~58us of traffic (4MB x16 + 2MB xt in, 4MB y out, per batch).
  - Global-shift symmetric exp: E = exp(s0 - S) with one scalar s0
    (global min of S) is symmetric, so fc_raw = E @ f1 takes its matmul
    lhsT directly from E's stored row-tiles. The per-row softmax scale
    folds into the epilogue: y = (beta / Zraw[c]) * fc_raw + x with
    Zraw = rowsum(E); the s0 and row-min shifts cancel exactly.
  - fp8e4 matmuls in DoubleRow perf mode for both S and fc; fp32 PSUM.
  - x is staged host-side to fp16; f8q (fc rhs) is cast on-device from
    x16 in [128,1024] pieces split across ACT/DVE/Pool.
  - beta-robust: Zraw is clamped before the reciprocal and br4 is
    written through a beta!=0-predicated copy, so beta=0 yields exactly
    y = x even if a degenerate row overflowed the softmax normalizer.
  - All HBM DMA issues from the SP sequencer via HWDGE. Queue order:
    [x16 b0][xt b0][x16 b1][xt b1][stores b0][stores b1] — the DMA
    stream is the bottleneck and never idles; compute hides behind it.
    PE order: S(0), fc(0), S(1), fc(1); fc alternates the S-pool and
    fc-pool PSUM banks (8 total) so bank recycling never stalls the PE.
"""

import numpy as np

B, C, HW = 16, 512, 4096
NCORES = 8
BL = B // NCORES  # batches per core
P = 128
CT = C // P       # 4 c-tiles of 128 channels
F = 512           # psum free dim / fc n-chunk
NQ = HW // 4      # 1024: store/cast granularity
HALF = HW // 2    # 2048
KTH = 16          # k-tiles per half

_CACHE = {}
_PHASES = []  # (label, next-instruction marker) for offline timeline analysis


def _build():
    import concourse.bass as bass  # noqa: F401
    import concourse.mybir as mybir
    import concourse.tile as tile
    from concourse import bacc, bass_isa

    f32 = mybir.dt.float32
    f16 = mybir.dt.float16
    f8 = mybir.dt.float8e4
    AF = mybir.ActivationFunctionType
    OP = mybir.AluOpType
    AX = mybir.AxisListType
    DR = mybir.MatmulPerfMode.DoubleRow

    nc = bacc.Bacc("TRN2", target_bir_lowering=False, debug=False)
    x_d = nc.dram_tensor("x", [BL, C, HW], f16, kind="ExternalInput")
    xt_d = nc.dram_tensor("xt", [BL, 2, P, KTH, F], f8, kind="ExternalInput")
    beta_d = nc.dram_tensor("beta", [1], f32, kind="ExternalInput")
    y_d = nc.dram_tensor("y", [BL, C, HW], f16, kind="ExternalOutput")

    def _mark(label):
        _PHASES.append((label, nc.get_next_instruction_name()))

    with tile.TileContext(nc) as tc:
        with (
            tc.tile_pool(name="singles", bufs=1) as singles,
            tc.tile_pool(name="x16", bufs=8) as x16_p,      # [128,4096] f16
            tc.tile_pool(name="f1t", bufs=4) as f1t_p,      # [128,16,512] f8
            tc.tile_pool(name="f8", bufs=4) as f8_p,        # [128,2,4096] f8
            tc.tile_pool(name="e2", bufs=4) as e2_p,        # [128,2,512] f8
            tc.tile_pool(name="soft", bufs=28) as soft_p,   # [128,<=4] f32
            tc.tile_pool(name="outs", bufs=16) as out_p,    # [128,2048] f16
            tc.tile_pool(name="tmps", bufs=8) as tmp_p,     # [128,512] f16
            tc.tile_pool(name="ps_s", bufs=4, space="PSUM") as ps_s,
            tc.tile_pool(name="ps_fc", bufs=4, space="PSUM") as ps_fc,
        ):
            beta_sb = singles.tile([P, 1], f32)
            beta_nz = singles.tile([P, 1], mybir.dt.uint8)
            ones8 = singles.tile([P, 1], f8)
            nc.gpsimd.memset(ones8[:], 1.0)

            state = {}

            def new_state(b):
                state[b] = {
                    "x16": [
                        x16_p.tile([P, HW], f16, tag="x16", name=f"x_{b}_{ct}")
                        for ct in range(CT)
                    ],
                    "f8q": [
                        f8_p.tile([P, 2, HW], f8, tag="f8", name=f"f8_{b}_{q}")
                        for q in range(2)
                    ],
                    "f1t": [
                        f1t_p.tile([P, KTH, F], f8, tag="f1t", name=f"f1t_{b}_{h}")
                        for h in range(2)
                    ],
                    "s_ps": [
                        ps_s.tile([P, F], f32, tag="s", name=f"s_ps_{b}_{m}")
                        for m in range(CT)
                    ],
                    "br4": soft_p.tile([P, CT], f32, tag="sm", name=f"br4_{b}"),
                }
                nc.gpsimd.memset(state[b]["br4"][:], 0.0)

            def emit_loads(b):
                _mark(f'loads{b}')
                # f1t k-slabs first (gate S), then x16 (casts/residual)
                x16, f1t = state[b]["x16"], state[b]["f1t"]
                for h in range(2):
                    for ks in range(2):
                        nc.sync.dma_start(
                            out=f1t[h][:, ks * 8 : (ks + 1) * 8, :],
                            in_=xt_d[b, h, :, ks * 8 : (ks + 1) * 8, :],
                        )
                for ct in range(CT):
                    nc.sync.dma_start(
                        out=x16[ct][:], in_=x_d[b, ct * P : (ct + 1) * P, :]
                    )

            def emit_casts(b, cts, engs, qs=(0, 1, 2, 3)):
                _mark(f'cast{b}_{cts[0]}')
                # fp16 -> fp8 casts in [128,1024] pieces on the given engines
                x16, f8q = state[b]["x16"], state[b]["f8q"]
                i = 0
                for ct in cts:
                    for q in qs:
                        dst = f8q[ct // 2][:, ct % 2, q * NQ : (q + 1) * NQ]
                        src = x16[ct][:, q * NQ : (q + 1) * NQ]
                        engs[i % len(engs)](out=dst, in_=src)
                        i += 1

            def emit_S_h0(b):
                _mark(f'Sh0_{b}')
                f1t, s_ps = state[b]["f1t"], state[b]["s_ps"]
                for q in range(8):
                    for m in range(CT):
                        nc.tensor.matmul(
                            s_ps[m][:],
                            lhsT=f1t[0][:, 2 * q : 2 * q + 2, m * P : (m + 1) * P],
                            rhs=f1t[0][:, 2 * q : 2 * q + 2, :],
                            start=(q == 0),
                            stop=False,
                            perf_mode=DR,
                        )

            def emit_S_h1_stats(b):
                _mark(f'Sh1_{b}')
                # m-outer; each row-min fires as its m-tile stops and feeds
                # that m-tile's exp directly (per-row shift — no global-min
                # chain; the row weights e^{s_d} cancel between fc_raw and
                # the column-sum normalizer Zcol)
                f1t, s_ps = state[b]["f1t"], state[b]["s_ps"]
                nm4 = soft_p.tile([P, CT], f32, tag="sm", name=f"nm4_{b}")
                for m in range(CT):
                    for q in range(8):
                        nc.tensor.matmul(
                            s_ps[m][:],
                            lhsT=f1t[1][:, 2 * q : 2 * q + 2, m * P : (m + 1) * P],
                            rhs=f1t[1][:, 2 * q : 2 * q + 2, :],
                            start=False,
                            stop=(q == 7),
                            perf_mode=DR,
                        )
                    nc.vector.tensor_reduce(
                        out=nm4[:, m : m + 1],
                        in_=s_ps[m][:],
                        axis=AX.X,
                        op=OP.min,
                    )
                state[b]["nm4"] = nm4

            def emit_exps(b):
                _mark(f'exp{b}')
                # E[d,:] = exp(s_d - S[d,:]) in fp8, s_d = row-min: each exp
                # needs only its own m-tile's min, so the chain pipelines
                # with the S h1 matmuls instead of waiting on a global stat
                s_ps, nm4 = state[b]["s_ps"], state[b]["nm4"]
                e2 = [
                    e2_p.tile([P, 2, F], f8, tag="e2", name=f"e2_{b}_{g}")
                    for g in range(2)
                ]
                for m in range(CT):
                    nc.scalar.activation(
                        out=e2[m // 2][:, m % 2, :],
                        in_=s_ps[m][:],
                        func=AF.Exp,
                        bias=nm4[:, m : m + 1],
                        scale=-1.0,
                    )
                state[b]["e2"] = e2

            def emit_zcol_br(b):
                _mark(f'br{b}')
                # Zcol[c] = sum_d E[d,c] via 16 tiny column-sum matmuls
                # against a ones vector (free size 1 — near-zero PE time),
                # accumulated per c-block in a [128,CT] PSUM tile. Then
                # beta / Zcol, finite for any beta: clamp before the
                # reciprocal, and write br4 through a beta-predicated copy
                # so br4 is exactly 0.0 (not 0*NaN) for beta == 0.
                e2 = state[b]["e2"]
                zps = ps_s.tile([P, F], f32, tag="s", name=f"zps_{b}")
                for m in range(CT):
                    for db in range(CT):
                        nc.tensor.matmul(
                            zps[:, m : m + 1],
                            lhsT=e2[db // 2][:, db % 2, m * P : (m + 1) * P],
                            rhs=ones8[:],
                            start=(db == 0),
                            stop=(db == 3),
                        )
                zs = soft_p.tile([P, CT], f32, tag="sm", name=f"zs_{b}")
                nc.vector.tensor_scalar(
                    out=zs[:], in0=zps[:, 0:CT], scalar1=1e-35, scalar2=None, op0=OP.max
                )
                rz = soft_p.tile([P, CT], f32, tag="sm", name=f"rz_{b}")
                nc.vector.reciprocal(out=rz[:], in_=zs[:])
                rzb = soft_p.tile([P, CT], f32, tag="sm", name=f"rzb_{b}")
                nc.gpsimd.tensor_scalar_mul(out=rzb[:], in0=rz[:], scalar1=beta_sb[:])
                br4 = state[b]["br4"]
                nc.vector.copy_predicated(
                    out=br4[:],
                    mask=beta_nz[:].broadcast_to([P, CT]),
                    data=rzb[:],
                )

            def emit_fc_quarter(b, nq, cnt, ot):
                _mark(f'fc{b}_{nq}')
                # fc_raw = E @ f1 for quarter nq; the qe=0 sweep over all 8
                # chunks runs first (needs only exps 0-1 / f8q c-tiles 0-1),
                # then the qe=1 sweep stops each chunk — pipelines fc into
                # the exp chain and the b1 cast arrivals. 8 chunks hold all
                # 8 PSUM banks (4 ps_s + 4 ps_fc); fused epilogue
                # y = (beta/Zraw)[c]*fc_raw + x follows each chunk's stop.
                x16, f8q = state[b]["x16"], state[b]["f8q"]
                e2, br4 = state[b]["e2"], state[b]["br4"]
                h, qo = nq // 2, (nq % 2) * NQ
                chunks = []
                for m in range(CT):
                    if nq % 2 == 0:
                        ot[(m, h)] = out_p.tile(
                            [P, HALF], f16, tag="out", name=f"ot_{b}_{h}_{m}"
                        )
                    for jj in range(2):
                        j = nq * 2 + jj
                        c = cnt[0]
                        # b0 front-loads its ps_s uses (and avoids them in
                        # its last two quarters) so the slots recycle to
                        # batch 1's S accumulators without stalling it
                        if b == 0 and nq >= 2:
                            use_s = False
                        else:
                            use_s = (c % 2 == 0) if b == 0 else (c % 2 == 1)
                        pool = ps_s if use_s else ps_fc
                        tag = "s" if pool is ps_s else "fc"
                        f_ps = pool.tile([P, F], f32, tag=tag, name=f"f_{b}_{j}_{m}")
                        chunks.append((m, jj, j, c, f_ps))
                        cnt[0] += 1
                # qe-split needs all 8 PSUM banks live at once; a quarter
                # restricted to one 4-bank pool must complete chunk-by-chunk
                # (qe-inner) or the in-order PE queue deadlocks on recycling
                split = not (b == 0 and nq >= 2)
                if split:
                    for qe in range(2):
                        for m, jj, j, c, f_ps in chunks:
                            nc.tensor.matmul(
                                f_ps[:],
                                lhsT=e2[qe][:, :, m * P : (m + 1) * P],
                                rhs=f8q[qe][:, :, j * F : (j + 1) * F],
                                start=(qe == 0),
                                stop=(qe == 1),
                                perf_mode=DR,
                            )
                else:
                    for m, jj, j, c, f_ps in chunks:
                        for qe in range(2):
                            nc.tensor.matmul(
                                f_ps[:],
                                lhsT=e2[qe][:, :, m * P : (m + 1) * P],
                                rhs=f8q[qe][:, :, j * F : (j + 1) * F],
                                start=(qe == 0),
                                stop=(qe == 1),
                                perf_mode=DR,
                            )
                for m, jj, j, c, f_ps in chunks:
                    o = ot[(m, h)]
                    oslice = o[:, qo + jj * F : qo + (jj + 1) * F]
                    xslice = x16[m][:, nq * NQ + jj * F : nq * NQ + (jj + 1) * F]
                    kind = ((0, 2, 0, 1) if b == 0 else (2, 0, 1, 0))[c % 4]
                    if kind:
                        # PSUM can only be read by ACT/DVE: ACT scales
                        # fc out of PSUM, the fp16 residual add runs on
                        # DVE (2x) or Pool (all-SBUF)
                        tmp = tmp_p.tile([P, F], f16, tag="tmp", name=f"tp_{b}_{j}_{m}")
                        nc.scalar.mul(out=tmp[:], in_=f_ps[:], mul=br4[:, m : m + 1])
                        aeng = nc.vector if kind == 1 else nc.gpsimd
                        aeng.tensor_tensor(
                            out=oslice, in0=tmp[:], in1=xslice, op=OP.add
                        )
                    else:
                        nc.vector.scalar_tensor_tensor(
                            out=oslice,
                            in0=f_ps[:],
                            scalar=br4[:, m : m + 1],
                            in1=xslice,
                            op0=OP.mult,
                            op1=OP.add,
                        )
                    if jj == 1:
                        nc.sync.dma_start(
                            out=y_d[b, m * P : (m + 1) * P, nq * NQ : (nq + 1) * NQ],
                            in_=o[:, qo : qo + NQ],
                        )

            # ---- program order: DMA queue [xt b0][x16 b0][xt b1][x16 b1]
            # ---- [stores b0][stores b1]; PE queue [warmup][S0][fc0][S1][fc1]
            new_state(0)
            emit_loads(0)
            # beta rides the DMA queue behind b0's first slabs (tiny; keeps
            # the first data transfer at the queue head)
            nc.sync.dma_start(out=beta_sb[:], in_=beta_d[:].to_broadcast([P, 1]))
            nc.vector.tensor_scalar(
                out=beta_nz[:], in0=beta_sb[:], scalar1=0.0, scalar2=None,
                op0=OP.not_equal,
            )
            new_state(1)
            emit_loads(1)
            # PE p-state pre-warm: ungated dummy matmuls run back-to-back
            # from t~0.4us so S(0) starts at full clock when its first f1t
            # slab lands (~4us). The dummy reads a memset tile; its PSUM
            # slot recycles into the first fc chunks much later.
            _mark('warm')
            wsrc = singles.tile([P, 2, F], f8)
            nc.gpsimd.memset(wsrc[:], 0.0)
            wps = ps_fc.tile([P, F], f32, tag="fc", name="warm_ps")
            for i in range(16):
                nc.tensor.matmul(
                    wps[:],
                    lhsT=wsrc[:, :, 0:P],
                    rhs=wsrc[:],
                    start=(i == 0),
                    stop=(i == 15),
                    perf_mode=DR,
                )
            # b0: stats chain ahead of casts on DVE (row-mins must not sit
            # behind cast pieces gated on later x16 c-tile arrivals)
            emit_S_h0(0)
            emit_S_h1_stats(0)
            emit_casts(0, (0, 1), [nc.vector.tensor_copy, nc.gpsimd.tensor_copy])
            emit_exps(0)
            emit_zcol_br(0)
            emit_casts(
                0, (2, 3),
                [nc.scalar.copy, nc.vector.tensor_copy,
                 nc.gpsimd.tensor_copy, nc.scalar.copy],
            )
            cnt0, ot0 = [0], {}
            emit_fc_quarter(0, 0, cnt0, ot0)
            emit_fc_quarter(0, 1, cnt0, ot0)
            emit_fc_quarter(0, 2, cnt0, ot0)
            # S(1)-h0 emitted before fc0's last quarter: its matmuls fill
            # the PE stalls left by fc0's epilogue-paced bank recycling
            # (the s_ps-b1 slots are free once exps(0) has read them)
            emit_S_h0(1)
            emit_fc_quarter(0, 3, cnt0, ot0)
            # b1 casts follow the x16-b1 c-tile arrivals: ACT/Pool take
            # ct0/1 (DVE still owns fc0's epilogue, then the row-min rail),
            # DVE picks up ct2/3 after the mins; exps(1) slots before the
            # last ACT piece
            A, V, G = nc.scalar.copy, nc.vector.tensor_copy, nc.gpsimd.tensor_copy
            emit_casts(1, (0,), [A, G, A, G])
            emit_casts(1, (1,), [A, G, A, G])
            emit_S_h1_stats(1)
            emit_casts(1, (2,), [V, V, G, A])
            emit_casts(1, (3,), [V, V, G], qs=(0, 1, 2))
            emit_exps(1)
            emit_zcol_br(1)
            emit_casts(1, (3,), [A], qs=(3,))
            cnt1, ot1 = [0], {}
            for nq in range(4):
                emit_fc_quarter(1, nq, cnt1, ot1)
    nc.finalize()
    return nc


def _get_nc():
    if "nc" not in _CACHE:
        _CACHE["nc"] = _build()
    return _CACHE["nc"]


def kernel(x: np.ndarray, beta: np.ndarray, **kw) -> np.ndarray:
    import concourse.mybir as mybir
    from concourse.bass_utils import run_bass_kernel_spmd

    x = np.asarray(x)
    beta = np.ascontiguousarray(np.asarray(beta, dtype=np.float32))
    assert x.shape == (B, C, 64, 64), x.shape

    f8np = mybir.dt.np(mybir.dt.float8e4)
    x16 = np.ascontiguousarray(x.reshape(B, C, HW).astype(np.float16))
    # xt[b, h, p, k, c] = fp8(x16[b, c, h*2048 + k*128 + p])
    xt = np.ascontiguousarray(
        x16.reshape(B, C, 2, KTH, P).transpose(0, 2, 4, 3, 1).astype(f8np)
    )
    in_maps = [
        {
            "x": np.ascontiguousarray(x16[i * BL : (i + 1) * BL]),
            "xt": np.ascontiguousarray(xt[i * BL : (i + 1) * BL]),
            "beta": beta,
        }
        for i in range(NCORES)
    ]
    nc = _get_nc()
    res = run_bass_kernel_spmd(nc, in_maps, core_ids=list(range(NCORES)))
    out = np.concatenate([r["y"] for r in res.results], axis=0)
    return out.reshape(B, C, 64, 64).astype(np.float32)
